# revision 8
# baseline (speedup 1.0000x reference)
"""DMPNN message-passing kernel for 8 Trainium2 NeuronCores.

Sharding: destination-node (column) shard. Core k owns output nodes
C_k = [k*512, (k+1)*512). It holds adj[:, C_k] and edge_features[:, C_k, :],
so the edge aggregates (esum, deg) are fully local; only h (16 x 512 bf16,
16 KB) is all-gathered between layers.

Per-core device program:
  - adj (bf16, exact 0/1) resident in SBUF [128, 32*512].
  - edge features streamed (3 planar components, bf16), masked with adj on
    the vector engine (bf16 2x mode), reduced over source nodes with
    ones-vector matmuls accumulating in PSUM -> esumT [3,512], degT [1,512].
  - per layer: msgT[16,512] = sum_b hW_b^T @ adj_b  (+ We^T esum + Wb deg),
    x = h_loc + msg, h'_loc = U x + U_b; AllGather h' -> full h (bf16);
    hW_{l+1} computed on PE from gathered h.
  - output: per-core column sum of final h [16]; readout done on host.
"""

import os
import numpy as np
import ml_dtypes

N = 4096
HID = 16
EDIM = 3
LAYERS = 3
NCORES = 8
J = N // NCORES          # 512 columns owned per core
NB = N // 128            # 32 source-node blocks of 128
CHUNKS = 8               # stream chunks (4 blocks each)
BPC = NB // CHUNKS       # blocks per chunk
CW = BPC * 512           # chunk width in free elements (2048)

BF16 = ml_dtypes.bfloat16

# Results of the last traced run (read by test.py)
LAST_EXEC_TIME_NS = None
LAST_RESULTS = None

_CACHE = {}


def _build_module():
    import concourse.bacc as bacc
    import concourse.tile as tile
    import concourse.mybir as mybir

    f32 = mybir.dt.float32
    bf16 = mybir.dt.bfloat16
    AX = mybir.AxisListType
    OP = mybir.AluOpType

    nc = bacc.Bacc("TRN2", target_bir_lowering=False, debug=False,
                   num_devices=NCORES)

    adj_d = nc.dram_tensor("adj", [CHUNKS, 128, CW], bf16, kind="ExternalInput")
    ef_d = nc.dram_tensor("ef", [EDIM, CHUNKS, 128, CW], bf16, kind="ExternalInput")
    h0loc_d = nc.dram_tensor("h0loc", [HID, J], bf16, kind="ExternalInput")
    hw0_d = nc.dram_tensor("hw0", [128, NB * HID], bf16, kind="ExternalInput")
    cb_d = nc.dram_tensor("cb", [HID, 3 * HID], bf16, kind="ExternalInput")
    cf_d = nc.dram_tensor("cf", [HID, 18 * HID], f32, kind="ExternalInput")
    out_d = nc.dram_tensor("out_p", [HID, 1], f32, kind="ExternalOutput")

    rg = [list(range(NCORES))]

    with tile.TileContext(nc) as tc:
        with (
            tc.tile_pool(name="const", bufs=1) as const,
            tc.tile_pool(name="hwp", bufs=1) as hwp,
            tc.tile_pool(name="estream", bufs=4) as estream,
            tc.tile_pool(name="mpool", bufs=4) as mpool,
            tc.tile_pool(name="small", bufs=1) as small,
            tc.tile_pool(name="hbuf", bufs=2) as hbuf,
            tc.tile_pool(name="ps_acc", bufs=1, space="PSUM") as ps_acc,
            tc.tile_pool(name="ps_msg", bufs=1, space="PSUM") as ps_msg,
            tc.tile_pool(name="ps_h", bufs=1, space="PSUM") as ps_h,
            tc.tile_pool(name="ps_hw", bufs=1, space="PSUM") as ps_hw,
            tc.tile_pool(name="dram", bufs=2, space="DRAM") as dram,
        ):
            # --- constants / small inputs ---
            cb_t = const.tile([HID, 3 * HID], bf16)
            nc.sync.dma_start(cb_t[:], cb_d[:])
            cf_t = const.tile([HID, 18 * HID], f32)
            nc.sync.dma_start(cf_t[:], cf_d[:])
            h0loc_t = const.tile([HID, J], bf16)
            nc.sync.dma_start(h0loc_t[:], h0loc_d[:])
            hw0_t = hwp.tile([128, NB * HID], bf16, tag="hw0")
            nc.sync.dma_start(hw0_t[:], hw0_d[:])
            ones_col = const.tile([128, 1], bf16)
            nc.vector.memset(ones_col[:], 1.0)
            ones_row = const.tile([1, J], f32)
            nc.vector.memset(ones_row[:], 1.0)

            adj_t = const.tile([128, NB * 512], bf16)

            # cf layout (fp32): UT_l @ [0:16, 16l), Wb_l @ [0:1, 48+16l),
            # U_b_l @ [0:1, 96+16l), We_l[:, c] (row vec) @ [0:1, 144+(3l+c)*16)
            def UT(l):
                return cf_t[0:HID, 16 * l:16 * l + 16]

            def Wb(l):
                return cf_t[0:1, 48 + 16 * l:64 + 16 * l]

            def Ub(l):
                return cf_t[0:1, 96 + 16 * l:112 + 16 * l]

            def WeRow(l, c):
                o = 144 + (3 * l + c) * 16
                return cf_t[0:1, o:o + 16]

            def WhT(l):
                return cb_t[:, 16 * l:16 * l + 16]

            # --- phase A: stream edges, build esum/deg; start layer-0 msg ---
            esum_ps = [
                ps_acc.tile([1, 512], f32, tag=f"esum{c}", name=f"esum_ps{c}")
                for c in range(EDIM)
            ]
            deg_ps = ps_acc.tile([1, 512], f32, tag="deg")
            msg_ps = ps_msg.tile([HID, 512], f32, tag="msg")

            hw_tiles = [hw0_t, None, None]

            for ch in range(CHUNKS):
                csl = slice(ch * CW, (ch + 1) * CW)
                nc.sync.dma_start(adj_t[:, csl], adj_d[ch])
                for c in range(EDIM):
                    e_t = estream.tile([128, CW], bf16)
                    nc.sync.dma_start(e_t[:], ef_d[c, ch])
                    m_t = mpool.tile([128, CW], bf16)
                    nc.vector.tensor_tensor(m_t[:], adj_t[:, csl], e_t[:], OP.mult)
                    for bb in range(BPC):
                        b = ch * BPC + bb
                        nc.tensor.matmul(
                            esum_ps[c][:], ones_col[:],
                            m_t[:, bb * 512:(bb + 1) * 512],
                            start=(b == 0), stop=(b == NB - 1),
                        )
                for bb in range(BPC):
                    b = ch * BPC + bb
                    nc.tensor.matmul(
                        deg_ps[:], ones_col[:],
                        adj_t[:, b * 512:(b + 1) * 512],
                        start=(b == 0), stop=(b == NB - 1),
                    )
                for bb in range(BPC):
                    b = ch * BPC + bb
                    nc.tensor.matmul(
                        msg_ps[:], hw0_t[:, b * HID:(b + 1) * HID],
                        adj_t[:, b * 512:(b + 1) * 512],
                        start=(b == 0), stop=False,
                    )

            esum_sb = [
                small.tile([1, 512], f32, tag=f"esum_sb{c}", name=f"esum_sb{c}")
                for c in range(EDIM)
            ]
            for c in range(EDIM):
                nc.scalar.copy(esum_sb[c][:], esum_ps[c][:])
            degT = small.tile([1, 512], f32)
            nc.scalar.copy(degT[:], deg_ps[:])

            # --- phase B: layers ---
            hprev_loc = h0loc_t
            for l in range(LAYERS):
                if l > 0:
                    msg_ps = ps_msg.tile([HID, 512], f32, tag="msg")
                    for b in range(NB):
                        nc.tensor.matmul(
                            msg_ps[:], hw_tiles[l][:, b * HID:(b + 1) * HID],
                            adj_t[:, b * 512:(b + 1) * 512],
                            start=(b == 0), stop=False,
                        )
                for c in range(EDIM):
                    nc.tensor.matmul(msg_ps[:], WeRow(l, c), esum_sb[c][:],
                                     start=False, stop=False)
                nc.tensor.matmul(msg_ps[:], Wb(l), degT[:], start=False, stop=True)

                xT = small.tile([HID, 512], f32, tag="xT", bufs=2)
                nc.vector.tensor_tensor(xT[:], hprev_loc[:], msg_ps[:], OP.add)

                h_ps = ps_h.tile([HID, 512], f32, tag="h")
                nc.tensor.matmul(h_ps[:], UT(l), xT[:], start=True, stop=False)
                nc.tensor.matmul(h_ps[:], Ub(l), ones_row[:], start=False, stop=True)

                if l < LAYERS - 1:
                    hloc_f = small.tile([HID, 512], f32, tag="hlocf", bufs=2)
                    nc.vector.tensor_copy(hloc_f[:], h_ps[:])
                    hloc_b = small.tile([HID, 512], bf16, tag="hlocb", bufs=2)
                    nc.scalar.copy(hloc_b[:], h_ps[:])
                    cc_in = dram.tile([HID, 512], bf16, tag="cc_in")
                    nc.sync.dma_start(cc_in[:], hloc_b[:])
                    cc_out = dram.tile([NCORES * HID, 512], bf16, tag="cc_out")
                    nc.gpsimd.collective_compute(
                        "AllGather", mybir.AluOpType.bypass,
                        replica_groups=rg,
                        ins=[cc_in[:]], outs=[cc_out[:]],
                    )
                    hT_full = hbuf.tile([HID, N], bf16, tag="hfull")
                    nc.sync.dma_start(
                        hT_full[:].rearrange("h (r j) -> h r j", r=NCORES),
                        cc_out[:].rearrange("(r h) j -> h r j", r=NCORES),
                    )
                    hw_ps = ps_hw.tile([128, NB * HID], f32, tag="hw")
                    for b in range(NB):
                        nc.tensor.matmul(
                            hw_ps[:, b * HID:(b + 1) * HID],
                            hT_full[:, b * 128:(b + 1) * 128],
                            WhT(l + 1),
                            start=True, stop=True,
                        )
                    hw_next = hwp.tile([128, NB * HID], bf16, tag=f"hw{l + 1}")
                    nc.vector.tensor_copy(hw_next[:], hw_ps[:])
                    hw_tiles[l + 1] = hw_next
                    hprev_loc = hloc_f
                else:
                    gpart = small.tile([HID, 1], f32)
                    nc.vector.reduce_sum(gpart[:], h_ps[:], axis=AX.X)
                    nc.sync.dma_start(out_d[:], gpart[:])

    nc.compile()
    return nc


def _get_module():
    if "nc" not in _CACHE:
        _CACHE["nc"] = _build_module()
    return _CACHE["nc"]


def _shard_cols(a2d):
    """[4096, 512] fp32/bf16 -> [CHUNKS, 128, CW] bf16 (block-of-128 layout)."""
    a = np.asarray(a2d).astype(BF16)
    a = a.reshape(CHUNKS, BPC, 128, 512).transpose(0, 2, 1, 3)
    return np.ascontiguousarray(a.reshape(CHUNKS, 128, CW))


def kernel(node_features, edge_features, adj_matrix,
           emb_w, emb_b, W_h, W_e, W_b, U_w, U_b, ro_w, ro_b):
    global LAST_EXEC_TIME_NS, LAST_RESULTS
    from concourse.bass_utils import run_bass_kernel_spmd

    node_features = np.asarray(node_features, dtype=np.float32)
    edge_features = np.asarray(edge_features, dtype=np.float32)
    adj_matrix = np.asarray(adj_matrix, dtype=np.float32)
    W_h = np.asarray(W_h, dtype=np.float32)
    W_e = np.asarray(W_e, dtype=np.float32)
    W_b = np.asarray(W_b, dtype=np.float32)
    U_w = np.asarray(U_w, dtype=np.float32)
    U_b = np.asarray(U_b, dtype=np.float32)

    # tiny host-side preprocessing (node embedding + layer-0 hW)
    h0 = node_features @ np.asarray(emb_w, np.float32).T + np.asarray(emb_b, np.float32)
    hw0_full = h0 @ W_h[0].T                                   # [N, HID]
    hw0 = np.ascontiguousarray(
        hw0_full.reshape(NB, 128, HID).transpose(1, 0, 2).reshape(128, NB * HID)
    ).astype(BF16)

    cb = np.zeros((HID, 3 * HID), dtype=BF16)
    cf = np.zeros((HID, 18 * HID), dtype=np.float32)
    for l in range(LAYERS):
        cb[:, 16 * l:16 * l + 16] = W_h[l].T.astype(BF16)
        cf[0:HID, 16 * l:16 * l + 16] = U_w[l].T
        cf[0:1, 48 + 16 * l:64 + 16 * l] = W_b[l][None, :]
        cf[0:1, 96 + 16 * l:112 + 16 * l] = U_b[l][None, :]
        for c in range(EDIM):
            o = 144 + (3 * l + c) * 16
            cf[0:1, o:o + 16] = W_e[l][:, c][None, :]

    ebf = edge_features.astype(BF16)
    h0_bf = h0.astype(BF16)

    in_maps = []
    for k in range(NCORES):
        cols = slice(k * J, (k + 1) * J)
        adj_s = _shard_cols(adj_matrix[:, cols])
        ef_s = np.stack([_shard_cols(ebf[:, cols, c]) for c in range(EDIM)])
        h0loc = np.ascontiguousarray(h0_bf[cols, :].T)
        in_maps.append({
            "adj": adj_s, "ef": ef_s, "h0loc": h0loc,
            "hw0": hw0, "cb": cb, "cf": cf,
        })

    nc = _get_module()
    res = run_bass_kernel_spmd(nc, in_maps, core_ids=list(range(NCORES)))
    LAST_EXEC_TIME_NS = res.exec_time_ns
    LAST_RESULTS = res

    graph_rep = np.zeros(HID, dtype=np.float64)
    for k in range(NCORES):
        graph_rep += res.results[k]["out_p"][:, 0].astype(np.float64)
    out = graph_rep.astype(np.float32) @ np.asarray(ro_w, np.float32).T \
        + np.asarray(ro_b, np.float32)
    return out.astype(np.float32)


# revision 11
# speedup vs baseline: 1.1232x; 1.1232x over previous
"""DMPNN message-passing kernel for 8 Trainium2 NeuronCores.

Sharding: destination-node (column) shard. Core k owns output nodes
C_k = [k*512, (k+1)*512). It holds adj[:, C_k] and edge_features[:, C_k, :],
so the edge aggregates (esum, deg) are fully local; only h (16 x 512 bf16,
16 KB) is all-gathered between layers.

Per-core device program:
  - adj (bf16, exact 0/1) resident in SBUF [128, 32*512].
  - edge features streamed (3 planar components, bf16), masked with adj on
    the vector engine (bf16 2x mode), reduced over source nodes with
    ones-vector matmuls accumulating in PSUM -> esum[3][1,512].
  - deg is folded into the layer-0 message matmul: the stationary operand
    carries hW0 in columns 0..15 and a ones column at 32, so one pass over
    adj yields both msg0 (rows 0..15) and deg (row 32).
  - per layer: msgT[16,512] = sum_b hW_b^T @ adj_b (+ We^T esum + Wb deg),
    x = h_loc + msg, h'_loc = U x (+ U_b via ACT bias); AllGather h' ->
    full h (bf16); hW_{l+1} computed on PE from gathered h.
  - output: per-core column sum of final h [16] (U_b of the last layer is
    added on the host); readout done on host.
"""

import os
import numpy as np
import ml_dtypes

N = 4096
HID = 16
EDIM = 3
LAYERS = 3
NCORES = 8
J = N // NCORES          # 512 columns owned per core
NB = N // 128            # 32 source-node blocks of 128
CHUNKS = 8               # stream chunks (4 blocks each)
BPC = NB // CHUNKS       # blocks per chunk
CW = BPC * 512           # chunk width in free elements (2048)
M0 = 33                  # layer-0 stationary width: 16 hW + 16 pad + ones

BF16 = ml_dtypes.bfloat16

# Results of the last traced run (read by test.py)
LAST_EXEC_TIME_NS = None
LAST_RESULTS = None

_CACHE = {}


def _build_module():
    import concourse.bacc as bacc
    import concourse.tile as tile
    import concourse.mybir as mybir

    f32 = mybir.dt.float32
    bf16 = mybir.dt.bfloat16
    AX = mybir.AxisListType
    OP = mybir.AluOpType
    AF = mybir.ActivationFunctionType

    nc = bacc.Bacc("TRN2", target_bir_lowering=False, debug=False,
                   num_devices=NCORES)

    adj_d = nc.dram_tensor("adj", [CHUNKS, 128, CW], bf16, kind="ExternalInput")
    ef_d = nc.dram_tensor("ef", [EDIM, CHUNKS, 128, CW], bf16, kind="ExternalInput")
    h0loc_d = nc.dram_tensor("h0loc", [HID, J], bf16, kind="ExternalInput")
    hw0_d = nc.dram_tensor("hw0", [128, NB * M0], bf16, kind="ExternalInput")
    cb_d = nc.dram_tensor("cb", [HID, 18 * HID], bf16, kind="ExternalInput")
    cf_d = nc.dram_tensor("cf", [HID, LAYERS], f32, kind="ExternalInput")
    out_d = nc.dram_tensor("out_p", [HID, 1], f32, kind="ExternalOutput")

    rg = [list(range(NCORES))]

    with tile.TileContext(nc) as tc:
        with (
            tc.tile_pool(name="const", bufs=1) as const,
            tc.tile_pool(name="hwp", bufs=1) as hwp,
            tc.tile_pool(name="estream", bufs=4) as estream,
            tc.tile_pool(name="mpool", bufs=4) as mpool,
            tc.tile_pool(name="small", bufs=1) as small,
            tc.tile_pool(name="hbuf", bufs=2) as hbuf,
            tc.tile_pool(name="ps_acc", bufs=1, space="PSUM") as ps_acc,
            tc.tile_pool(name="ps_msg", bufs=1, space="PSUM") as ps_msg,
            tc.tile_pool(name="ps_h", bufs=1, space="PSUM") as ps_h,
            tc.tile_pool(name="ps_hw", bufs=1, space="PSUM") as ps_hw,
            tc.tile_pool(name="dram", bufs=2, space="DRAM") as dram,
        ):
            # --- constants / small inputs ---
            cb_t = const.tile([HID, 18 * HID], bf16)
            nc.sync.dma_start(cb_t[:], cb_d[:])
            cf_t = const.tile([HID, LAYERS], f32)
            nc.sync.dma_start(cf_t[:], cf_d[:])
            h0loc_t = const.tile([HID, J], bf16)
            nc.sync.dma_start(h0loc_t[:], h0loc_d[:])
            hw0_t = hwp.tile([128, NB * M0], bf16, tag="hw0")
            nc.sync.dma_start(hw0_t[:], hw0_d[:])
            ones_col = const.tile([128, 1], bf16)
            nc.vector.memset(ones_col[:], 1.0)

            adj_t = const.tile([128, NB * 512], bf16)

            # cb layout (bf16): WhT_l @ [0:16, 16l), UT_l @ [0:16, 48+16l),
            # Wb_l @ [0:1, 96+16l), We_l[:, c] (row) @ [0:1, 144+(3l+c)*16)
            def WhT(l):
                return cb_t[:, 16 * l:16 * l + 16]

            def UT(l):
                return cb_t[0:HID, 48 + 16 * l:64 + 16 * l]

            def Wb(l):
                return cb_t[0:1, 96 + 16 * l:112 + 16 * l]

            def WeRow(l, c):
                o = 144 + (3 * l + c) * 16
                return cb_t[0:1, o:o + 16]

            def UbCol(l):
                return cf_t[:, l:l + 1]

            # --- phase A: stream edges, esum + (msg0 with folded deg) ---
            esum_ps = [
                ps_acc.tile([1, 512], f32, tag=f"esum{c}", name=f"esum_ps{c}")
                for c in range(EDIM)
            ]
            msg_ps = ps_msg.tile([M0, 512], f32, tag="msg0")

            hw_tiles = [hw0_t, None, None]

            for ch in range(CHUNKS):
                csl = slice(ch * CW, (ch + 1) * CW)
                nc.sync.dma_start(adj_t[:, csl], adj_d[ch])
                for c in range(EDIM):
                    e_t = estream.tile([128, CW], bf16)
                    nc.sync.dma_start(e_t[:], ef_d[c, ch])
                    m_t = mpool.tile([128, CW], bf16)
                    nc.vector.tensor_tensor(m_t[:], adj_t[:, csl], e_t[:], OP.mult)
                    for bb in range(BPC):
                        b = ch * BPC + bb
                        nc.tensor.matmul(
                            esum_ps[c][:], ones_col[:],
                            m_t[:, bb * 512:(bb + 1) * 512],
                            start=(b == 0), stop=(b == NB - 1),
                        )
                for bb in range(BPC):
                    b = ch * BPC + bb
                    nc.tensor.matmul(
                        msg_ps[:], hw0_t[:, b * M0:(b + 1) * M0],
                        adj_t[:, b * 512:(b + 1) * 512],
                        start=(b == 0), stop=False,
                    )

            esum_sb = [
                small.tile([1, 512], bf16, tag=f"esum_sb{c}", name=f"esum_sb{c}")
                for c in range(EDIM)
            ]
            nc.scalar.copy(esum_sb[0][:], esum_ps[0][:])
            nc.vector.tensor_copy(esum_sb[1][:], esum_ps[1][:])
            nc.scalar.copy(esum_sb[2][:], esum_ps[2][:])
            degT = small.tile([1, 512], bf16)
            nc.vector.tensor_copy(degT[:], msg_ps[32:33, :])

            # --- phase B: layers ---
            hprev_loc = h0loc_t
            for l in range(LAYERS):
                if l > 0:
                    msg_ps = ps_msg.tile([HID, 512], f32, tag="msg")
                    for b in range(NB):
                        nc.tensor.matmul(
                            msg_ps[:], hw_tiles[l][:, b * HID:(b + 1) * HID],
                            adj_t[:, b * 512:(b + 1) * 512],
                            start=(b == 0), stop=False,
                        )
                for c in range(EDIM):
                    nc.tensor.matmul(msg_ps[0:HID, :], WeRow(l, c), esum_sb[c][:],
                                     start=False, stop=False)
                nc.tensor.matmul(msg_ps[0:HID, :], Wb(l), degT[:],
                                 start=False, stop=True)

                xT = small.tile([HID, 512], bf16, tag="xT", bufs=2)
                nc.vector.tensor_tensor(xT[:], hprev_loc[:], msg_ps[0:HID, :], OP.add)

                h_ps = ps_h.tile([HID, 512], f32, tag="h")
                nc.tensor.matmul(h_ps[:], UT(l), xT[:], start=True, stop=True)

                if l < LAYERS - 1:
                    hloc_f = small.tile([HID, 512], f32, tag="hlocf", bufs=2)
                    nc.scalar.activation(hloc_f[:], h_ps[:], AF.Identity,
                                         bias=UbCol(l), scale=1.0)
                    hloc_b = small.tile([HID, 512], bf16, tag="hlocb", bufs=2)
                    nc.vector.tensor_scalar_add(hloc_b[:], h_ps[:], UbCol(l))
                    cc_in = dram.tile([HID, 512], bf16, tag="cc_in")
                    nc.sync.dma_start(cc_in[:], hloc_b[:])
                    cc_out = dram.tile([NCORES * HID, 512], bf16, tag="cc_out",
                                       addr_space="Shared")
                    nc.gpsimd.collective_compute(
                        "AllGather", mybir.AluOpType.bypass,
                        replica_groups=rg,
                        ins=[cc_in[:]], outs=[cc_out[:]],
                    )
                    hT_full = hbuf.tile([HID, N], bf16, tag="hfull")
                    nc.sync.dma_start(
                        hT_full[:].rearrange("h (r j) -> h r j", r=NCORES),
                        cc_out[:].rearrange("(r h) j -> h r j", r=NCORES),
                    )
                    hw_ps = ps_hw.tile([128, NB * HID], f32, tag="hw")
                    for b in range(NB):
                        nc.tensor.matmul(
                            hw_ps[:, b * HID:(b + 1) * HID],
                            hT_full[:, b * 128:(b + 1) * 128],
                            WhT(l + 1),
                            start=True, stop=True,
                        )
                    hw_next = hwp.tile([128, NB * HID], bf16, tag=f"hw{l + 1}")
                    nc.vector.tensor_copy(hw_next[:], hw_ps[:])
                    hw_tiles[l + 1] = hw_next
                    hprev_loc = hloc_f
                else:
                    gpart = small.tile([HID, 1], f32)
                    nc.vector.reduce_sum(gpart[:], h_ps[:], axis=AX.X)
                    nc.sync.dma_start(out_d[:], gpart[:])

    nc.compile()
    return nc


def _get_module():
    if "nc" not in _CACHE:
        _CACHE["nc"] = _build_module()
    return _CACHE["nc"]


def _shard_cols(a2d):
    """[4096, 512] -> [CHUNKS, 128, CW] bf16 (block-of-128 layout)."""
    a = np.asarray(a2d).astype(BF16)
    a = a.reshape(CHUNKS, BPC, 128, 512).transpose(0, 2, 1, 3)
    return np.ascontiguousarray(a.reshape(CHUNKS, 128, CW))


def kernel(node_features, edge_features, adj_matrix,
           emb_w, emb_b, W_h, W_e, W_b, U_w, U_b, ro_w, ro_b):
    global LAST_EXEC_TIME_NS, LAST_RESULTS
    from concourse.bass_utils import run_bass_kernel_spmd

    node_features = np.asarray(node_features, dtype=np.float32)
    edge_features = np.asarray(edge_features, dtype=np.float32)
    adj_matrix = np.asarray(adj_matrix, dtype=np.float32)
    W_h = np.asarray(W_h, dtype=np.float32)
    W_e = np.asarray(W_e, dtype=np.float32)
    W_b = np.asarray(W_b, dtype=np.float32)
    U_w = np.asarray(U_w, dtype=np.float32)
    U_b = np.asarray(U_b, dtype=np.float32)

    # tiny host-side preprocessing (node embedding + layer-0 hW)
    h0 = node_features @ np.asarray(emb_w, np.float32).T + np.asarray(emb_b, np.float32)
    hw0_full = h0 @ W_h[0].T                                   # [N, HID]
    hw0 = np.zeros((NB, 128, M0), dtype=BF16)
    hw0[:, :, 0:HID] = hw0_full.reshape(NB, 128, HID).astype(BF16)
    hw0[:, :, 32] = 1.0
    hw0 = np.ascontiguousarray(hw0.transpose(1, 0, 2).reshape(128, NB * M0))

    cb = np.zeros((HID, 18 * HID), dtype=BF16)
    cf = np.zeros((HID, LAYERS), dtype=np.float32)
    for l in range(LAYERS):
        cb[:, 16 * l:16 * l + 16] = W_h[l].T.astype(BF16)
        cb[0:HID, 48 + 16 * l:64 + 16 * l] = U_w[l].T.astype(BF16)
        cb[0:1, 96 + 16 * l:112 + 16 * l] = W_b[l][None, :].astype(BF16)
        for c in range(EDIM):
            o = 144 + (3 * l + c) * 16
            cb[0:1, o:o + 16] = W_e[l][:, c][None, :].astype(BF16)
        cf[:, l] = U_b[l]

    ebf = edge_features.astype(BF16)
    h0_bf = h0.astype(BF16)

    in_maps = []
    for k in range(NCORES):
        cols = slice(k * J, (k + 1) * J)
        adj_s = _shard_cols(adj_matrix[:, cols])
        ef_s = np.stack([_shard_cols(ebf[:, cols, c]) for c in range(EDIM)])
        h0loc = np.ascontiguousarray(h0_bf[cols, :].T)
        in_maps.append({
            "adj": adj_s, "ef": ef_s, "h0loc": h0loc,
            "hw0": hw0, "cb": cb, "cf": cf,
        })

    nc = _get_module()
    res = run_bass_kernel_spmd(nc, in_maps, core_ids=list(range(NCORES)))
    LAST_EXEC_TIME_NS = res.exec_time_ns
    LAST_RESULTS = res

    graph_rep = np.zeros(HID, dtype=np.float64)
    for k in range(NCORES):
        graph_rep += res.results[k]["out_p"][:, 0].astype(np.float64)
    graph_rep += N * U_b[LAYERS - 1].astype(np.float64)  # last-layer bias
    out = graph_rep.astype(np.float32) @ np.asarray(ro_w, np.float32).T \
        + np.asarray(ro_b, np.float32)
    return out.astype(np.float32)


# revision 15
# speedup vs baseline: 1.1259x; 1.0024x over previous
"""DMPNN message-passing kernel for 8 Trainium2 NeuronCores.

Sharding: destination-node (column) shard. Core k owns output nodes
C_k = [k*512, (k+1)*512). It holds adj[:, C_k] and edge_features[:, C_k, :],
so the edge aggregates (esum, deg) are fully local; only h (16 x 512 bf16,
16 KB) is all-gathered between layers.

Per-core device program:
  - adj (bf16, exact 0/1) resident in SBUF [128, 32*512].
  - edge features streamed (3 planar components, bf16), masked with adj on
    the vector engine (bf16 2x mode), reduced over source nodes with
    ones-vector matmuls accumulating in PSUM -> esum[3][1,512].
  - deg is folded into the layer-0 message matmul: the stationary operand
    carries hW0 in columns 0..15 and a ones column at 32, so one pass over
    adj yields both msg0 (rows 0..15) and deg (row 32).
  - per layer: msgT[16,512] = sum_b hW_b^T @ adj_b (+ We^T esum + Wb deg),
    x = h_loc + msg, h'_loc = U x (+ U_b via ACT bias); AllGather h' ->
    full h (bf16); hW_{l+1} computed on PE from gathered h.
  - output: per-core column sum of final h [16] (U_b of the last layer is
    added on the host); readout done on host.
"""

import os
import numpy as np
import ml_dtypes

N = 4096
HID = 16
EDIM = 3
LAYERS = 3
NCORES = 8
J = N // NCORES          # 512 columns owned per core
NB = N // 128            # 32 source-node blocks of 128
CHUNKS = 8               # stream chunks (4 blocks each)
BPC = NB // CHUNKS       # blocks per chunk
CW = BPC * 512           # chunk width in free elements (2048)
M0 = 33                  # layer-0 stationary width: 16 hW + 16 pad + ones

BF16 = ml_dtypes.bfloat16

# Results of the last traced run (read by test.py)
LAST_EXEC_TIME_NS = None
LAST_RESULTS = None

_CACHE = {}


def _build_module():
    import concourse.bacc as bacc
    import concourse.tile as tile
    import concourse.mybir as mybir

    f32 = mybir.dt.float32
    bf16 = mybir.dt.bfloat16
    AX = mybir.AxisListType
    OP = mybir.AluOpType
    AF = mybir.ActivationFunctionType

    nc = bacc.Bacc("TRN2", target_bir_lowering=False, debug=False,
                   num_devices=NCORES)

    adj_d = nc.dram_tensor("adj", [CHUNKS, 128, CW], bf16, kind="ExternalInput")
    ef_d = nc.dram_tensor("ef", [CHUNKS, EDIM, 128, CW], bf16, kind="ExternalInput")
    h0loc_d = nc.dram_tensor("h0loc", [HID, J], bf16, kind="ExternalInput")
    hw0_d = nc.dram_tensor("hw0", [128, NB * M0], bf16, kind="ExternalInput")
    cb_d = nc.dram_tensor("cb", [HID, 18 * HID], bf16, kind="ExternalInput")
    cf_d = nc.dram_tensor("cf", [HID, LAYERS], f32, kind="ExternalInput")
    out_d = nc.dram_tensor("out_p", [HID, 1], f32, kind="ExternalOutput")

    rg = [list(range(NCORES))]

    with tile.TileContext(nc) as tc:
        with (
            tc.tile_pool(name="const", bufs=1) as const,
            tc.tile_pool(name="hwp", bufs=1) as hwp,
            tc.tile_pool(name="estream", bufs=3) as estream,
            tc.tile_pool(name="mpool", bufs=3) as mpool,
            tc.tile_pool(name="small", bufs=1) as small,
            tc.tile_pool(name="hbuf", bufs=2) as hbuf,
            tc.tile_pool(name="ps_acc", bufs=1, space="PSUM") as ps_acc,
            tc.tile_pool(name="ps_msg", bufs=1, space="PSUM") as ps_msg,
            tc.tile_pool(name="ps_h", bufs=1, space="PSUM") as ps_h,
            tc.tile_pool(name="ps_hw", bufs=1, space="PSUM") as ps_hw,
            tc.tile_pool(name="dram", bufs=2, space="DRAM") as dram,
        ):
            # --- constants / small inputs ---
            cb_t = const.tile([HID, 18 * HID], bf16)
            nc.sync.dma_start(cb_t[:], cb_d[:])
            cf_t = const.tile([HID, LAYERS], f32)
            nc.sync.dma_start(cf_t[:], cf_d[:])
            h0loc_t = const.tile([HID, J], bf16)
            nc.sync.dma_start(h0loc_t[:], h0loc_d[:])
            hw0_t = hwp.tile([128, NB * M0], bf16, tag="hw0")
            nc.sync.dma_start(hw0_t[:], hw0_d[:])
            ones_col = const.tile([128, 1], bf16)
            nc.vector.memset(ones_col[:], 1.0)

            adj_t = const.tile([128, NB * 512], bf16)

            # cb layout (bf16): WhT_l @ [0:16, 16l), UT_l @ [0:16, 48+16l),
            # Wb_l @ [0:1, 96+16l), We_l[:, c] (row) @ [0:1, 144+(3l+c)*16)
            def WhT(l):
                return cb_t[:, 16 * l:16 * l + 16]

            def UT(l):
                return cb_t[0:HID, 48 + 16 * l:64 + 16 * l]

            def Wb(l):
                return cb_t[0:1, 96 + 16 * l:112 + 16 * l]

            def WeRow(l, c):
                o = 144 + (3 * l + c) * 16
                return cb_t[0:1, o:o + 16]

            def UbCol(l):
                return cf_t[:, l:l + 1]

            # --- phase A: stream edges, esum + (msg0 with folded deg) ---
            esum_ps = [
                ps_acc.tile([1, 512], f32, tag=f"esum{c}", name=f"esum_ps{c}")
                for c in range(EDIM)
            ]
            msg_ps = ps_msg.tile([M0, 512], f32, tag="msg0")

            hw_tiles = [hw0_t, None, None]

            for ch in range(CHUNKS):
                csl = slice(ch * CW, (ch + 1) * CW)
                nc.sync.dma_start(adj_t[:, csl], adj_d[ch])
                e_t = estream.tile([128, EDIM * CW], bf16)
                nc.sync.dma_start(
                    e_t[:].rearrange("p (c w) -> p c w", c=EDIM),
                    ef_d[ch].rearrange("c p w -> p c w"),
                )
                m_t = mpool.tile([128, EDIM * CW], bf16)
                for c in range(EDIM):
                    wsl = slice(c * CW, (c + 1) * CW)
                    nc.vector.tensor_tensor(m_t[:, wsl], adj_t[:, csl],
                                            e_t[:, wsl], OP.mult)
                    for bb in range(BPC):
                        b = ch * BPC + bb
                        nc.tensor.matmul(
                            esum_ps[c][:], ones_col[:],
                            m_t[:, c * CW + bb * 512:c * CW + (bb + 1) * 512],
                            start=(b == 0), stop=(b == NB - 1),
                        )
                for bb in range(BPC):
                    b = ch * BPC + bb
                    nc.tensor.matmul(
                        msg_ps[:], hw0_t[:, b * M0:(b + 1) * M0],
                        adj_t[:, b * 512:(b + 1) * 512],
                        start=(b == 0), stop=False,
                    )

            esum_sb = [
                small.tile([1, 512], bf16, tag=f"esum_sb{c}", name=f"esum_sb{c}")
                for c in range(EDIM)
            ]
            nc.scalar.copy(esum_sb[0][:], esum_ps[0][:])
            nc.vector.tensor_copy(esum_sb[1][:], esum_ps[1][:])
            nc.scalar.copy(esum_sb[2][:], esum_ps[2][:])
            degT = small.tile([1, 512], bf16)
            nc.vector.tensor_copy(degT[:], msg_ps[32:33, :])

            # --- phase B: layers ---
            hprev_loc = h0loc_t
            for l in range(LAYERS):
                if l > 0:
                    msg_ps = ps_msg.tile([HID, 512], f32, tag="msg")
                    for b in range(NB):
                        nc.tensor.matmul(
                            msg_ps[:], hw_tiles[l][:, b * HID:(b + 1) * HID],
                            adj_t[:, b * 512:(b + 1) * 512],
                            start=(b == 0), stop=False,
                        )
                for c in range(EDIM):
                    nc.tensor.matmul(msg_ps[0:HID, :], WeRow(l, c), esum_sb[c][:],
                                     start=False, stop=False)
                nc.tensor.matmul(msg_ps[0:HID, :], Wb(l), degT[:],
                                 start=False, stop=True)

                xT = small.tile([HID, 512], bf16, tag="xT", bufs=2)
                nc.vector.tensor_tensor(xT[:], hprev_loc[:], msg_ps[0:HID, :], OP.add)

                h_ps = ps_h.tile([HID, 512], f32, tag="h")
                nc.tensor.matmul(h_ps[:], UT(l), xT[:], start=True, stop=True)

                if l < LAYERS - 1:
                    hloc_f = small.tile([HID, 512], f32, tag="hlocf", bufs=2)
                    nc.scalar.activation(hloc_f[:], h_ps[:], AF.Identity,
                                         bias=UbCol(l), scale=1.0)
                    hloc_b = small.tile([HID, 512], bf16, tag="hlocb", bufs=2)
                    nc.vector.tensor_scalar_add(hloc_b[:], h_ps[:], UbCol(l))
                    cc_in = dram.tile([HID, 512], bf16, tag="cc_in")
                    nc.sync.dma_start(cc_in[:], hloc_b[:])
                    cc_out = dram.tile([NCORES * HID, 512], bf16, tag="cc_out",
                                       addr_space="Shared")
                    nc.gpsimd.collective_compute(
                        "AllGather", mybir.AluOpType.bypass,
                        replica_groups=rg,
                        ins=[cc_in[:]], outs=[cc_out[:]],
                    )
                    hT_full = hbuf.tile([HID, N], bf16, tag="hfull")
                    nc.sync.dma_start(
                        hT_full[:].rearrange("h (r j) -> h r j", r=NCORES),
                        cc_out[:].rearrange("(r h) j -> h r j", r=NCORES),
                    )
                    hw_ps = ps_hw.tile([128, NB * HID], f32, tag="hw")
                    for b in range(NB):
                        nc.tensor.matmul(
                            hw_ps[:, b * HID:(b + 1) * HID],
                            hT_full[:, b * 128:(b + 1) * 128],
                            WhT(l + 1),
                            start=True, stop=True,
                        )
                    hw_next = hwp.tile([128, NB * HID], bf16, tag=f"hw{l + 1}")
                    nc.vector.tensor_copy(hw_next[:], hw_ps[:])
                    hw_tiles[l + 1] = hw_next
                    hprev_loc = hloc_f
                else:
                    gpart = small.tile([HID, 1], f32)
                    nc.vector.reduce_sum(gpart[:], h_ps[:], axis=AX.X)
                    nc.sync.dma_start(out_d[:], gpart[:])

    nc.compile()
    return nc


def _get_module():
    if "nc" not in _CACHE:
        _CACHE["nc"] = _build_module()
    return _CACHE["nc"]


def _shard_cols(a2d):
    """[4096, 512] -> [CHUNKS, 128, CW] bf16 (block-of-128 layout)."""
    a = np.asarray(a2d).astype(BF16)
    a = a.reshape(CHUNKS, BPC, 128, 512).transpose(0, 2, 1, 3)
    return np.ascontiguousarray(a.reshape(CHUNKS, 128, CW))


def kernel(node_features, edge_features, adj_matrix,
           emb_w, emb_b, W_h, W_e, W_b, U_w, U_b, ro_w, ro_b):
    global LAST_EXEC_TIME_NS, LAST_RESULTS
    from concourse.bass_utils import run_bass_kernel_spmd

    node_features = np.asarray(node_features, dtype=np.float32)
    edge_features = np.asarray(edge_features, dtype=np.float32)
    adj_matrix = np.asarray(adj_matrix, dtype=np.float32)
    W_h = np.asarray(W_h, dtype=np.float32)
    W_e = np.asarray(W_e, dtype=np.float32)
    W_b = np.asarray(W_b, dtype=np.float32)
    U_w = np.asarray(U_w, dtype=np.float32)
    U_b = np.asarray(U_b, dtype=np.float32)

    # tiny host-side preprocessing (node embedding + layer-0 hW)
    h0 = node_features @ np.asarray(emb_w, np.float32).T + np.asarray(emb_b, np.float32)
    hw0_full = h0 @ W_h[0].T                                   # [N, HID]
    hw0 = np.zeros((NB, 128, M0), dtype=BF16)
    hw0[:, :, 0:HID] = hw0_full.reshape(NB, 128, HID).astype(BF16)
    hw0[:, :, 32] = 1.0
    hw0 = np.ascontiguousarray(hw0.transpose(1, 0, 2).reshape(128, NB * M0))

    cb = np.zeros((HID, 18 * HID), dtype=BF16)
    cf = np.zeros((HID, LAYERS), dtype=np.float32)
    for l in range(LAYERS):
        cb[:, 16 * l:16 * l + 16] = W_h[l].T.astype(BF16)
        cb[0:HID, 48 + 16 * l:64 + 16 * l] = U_w[l].T.astype(BF16)
        cb[0:1, 96 + 16 * l:112 + 16 * l] = W_b[l][None, :].astype(BF16)
        for c in range(EDIM):
            o = 144 + (3 * l + c) * 16
            cb[0:1, o:o + 16] = W_e[l][:, c][None, :].astype(BF16)
        cf[:, l] = U_b[l]

    ebf = edge_features.astype(BF16)
    h0_bf = h0.astype(BF16)

    in_maps = []
    for k in range(NCORES):
        cols = slice(k * J, (k + 1) * J)
        adj_s = _shard_cols(adj_matrix[:, cols])
        ef_s = np.ascontiguousarray(np.stack(
            [_shard_cols(ebf[:, cols, c]) for c in range(EDIM)]
        ).transpose(1, 0, 2, 3))              # [CHUNKS, EDIM, 128, CW]
        h0loc = np.ascontiguousarray(h0_bf[cols, :].T)
        in_maps.append({
            "adj": adj_s, "ef": ef_s, "h0loc": h0loc,
            "hw0": hw0, "cb": cb, "cf": cf,
        })

    nc = _get_module()
    res = run_bass_kernel_spmd(nc, in_maps, core_ids=list(range(NCORES)))
    LAST_EXEC_TIME_NS = res.exec_time_ns
    LAST_RESULTS = res

    graph_rep = np.zeros(HID, dtype=np.float64)
    for k in range(NCORES):
        graph_rep += res.results[k]["out_p"][:, 0].astype(np.float64)
    graph_rep += N * U_b[LAYERS - 1].astype(np.float64)  # last-layer bias
    out = graph_rep.astype(np.float32) @ np.asarray(ro_w, np.float32).T \
        + np.asarray(ro_b, np.float32)
    return out.astype(np.float32)


# revision 17
# speedup vs baseline: 1.3294x; 1.1807x over previous
"""DMPNN message-passing kernel for 8 Trainium2 NeuronCores.

Sharding: destination-node (column) shard. Core k owns output nodes
C_k = [k*512, (k+1)*512). It holds adj[:, C_k] and edge_features[:, C_k, :],
so the edge aggregates (esum, deg) are fully local; only h (16 x 512 bf16,
16 KB) is all-gathered between layers.

Per-core device program:
  - adj (bf16, exact 0/1) resident in SBUF [128, 32*512].
  - edge features streamed (3 planar components, bf16), masked with adj on
    the vector engine (bf16 2x mode), reduced over source nodes with
    ones-vector matmuls accumulating in PSUM -> esum[3][1,512].
  - deg is folded into the layer-0 message matmul: the stationary operand
    carries hW0 in columns 0..15 and a ones column at 32, so one pass over
    adj yields both msg0 (rows 0..15) and deg (row 32).
  - per layer: msgT[16,512] = sum_b hW_b^T @ adj_b (+ We^T esum + Wb deg),
    x = h_loc + msg, h'_loc = U x (+ U_b via ACT bias); AllGather h' ->
    full h (bf16); hW_{l+1} computed on PE from gathered h.
  - output: per-core column sum of final h [16] (U_b of the last layer is
    added on the host); readout done on host.
"""

import os
import numpy as np
import ml_dtypes

N = 4096
HID = 16
EDIM = 3
LAYERS = 3
NCORES = 8
J = N // NCORES          # 512 columns owned per core
NB = N // 128            # 32 source-node blocks of 128
CHUNKS = 8               # stream chunks (4 blocks each)
BPC = NB // CHUNKS       # blocks per chunk
CW = BPC * 512           # chunk width in free elements (2048)
M0 = 33                  # layer-0 stationary width: 16 hW + 16 pad + ones

BF16 = ml_dtypes.bfloat16

# Results of the last traced run (read by test.py)
LAST_EXEC_TIME_NS = None
LAST_RESULTS = None

_CACHE = {}


def _build_module():
    import concourse.bacc as bacc
    import concourse.tile as tile
    import concourse.mybir as mybir

    f32 = mybir.dt.float32
    bf16 = mybir.dt.bfloat16
    AX = mybir.AxisListType
    OP = mybir.AluOpType
    AF = mybir.ActivationFunctionType

    nc = bacc.Bacc("TRN2", target_bir_lowering=False, debug=False,
                   num_devices=NCORES)

    adj_d = nc.dram_tensor("adj", [CHUNKS, 128, CW], bf16, kind="ExternalInput")
    ef_d = nc.dram_tensor("ef", [CHUNKS, EDIM, 128, CW], bf16, kind="ExternalInput")
    h0loc_d = nc.dram_tensor("h0loc", [HID, J], bf16, kind="ExternalInput")
    hw0_d = nc.dram_tensor("hw0", [128, NB * M0], bf16, kind="ExternalInput")
    cb_d = nc.dram_tensor("cb", [HID, 18 * HID], bf16, kind="ExternalInput")
    cf_d = nc.dram_tensor("cf", [HID, LAYERS], f32, kind="ExternalInput")
    out_d = nc.dram_tensor("out_p", [HID, 1], f32, kind="ExternalOutput")

    rg = [list(range(NCORES))]

    with tile.TileContext(nc) as tc:
        with (
            tc.tile_pool(name="const", bufs=1) as const,
            tc.tile_pool(name="hwp", bufs=1) as hwp,
            tc.tile_pool(name="estream", bufs=3) as estream,
            tc.tile_pool(name="mpool", bufs=3) as mpool,
            tc.tile_pool(name="small", bufs=1) as small,
            tc.tile_pool(name="hbuf", bufs=2) as hbuf,
            tc.tile_pool(name="ps_acc", bufs=1, space="PSUM") as ps_acc,
            tc.tile_pool(name="ps_msg", bufs=1, space="PSUM") as ps_msg,
            tc.tile_pool(name="ps_h", bufs=1, space="PSUM") as ps_h,
            tc.tile_pool(name="ps_hw", bufs=1, space="PSUM") as ps_hw,
            tc.tile_pool(name="dram", bufs=2, space="DRAM") as dram,
        ):
            # --- constants / small inputs ---
            cb_t = const.tile([HID, 18 * HID], bf16)
            nc.sync.dma_start(cb_t[:], cb_d[:])
            cf_t = const.tile([HID, LAYERS], f32)
            nc.sync.dma_start(cf_t[:], cf_d[:])
            h0loc_t = const.tile([HID, J], bf16)
            nc.sync.dma_start(h0loc_t[:], h0loc_d[:])
            hw0_t = hwp.tile([128, NB * M0], bf16, tag="hw0")
            nc.sync.dma_start(hw0_t[:], hw0_d[:])
            ones_col = const.tile([128, 1], bf16)
            nc.vector.memset(ones_col[:], 1.0)

            adj_t = const.tile([128, NB * 512], bf16)

            # cb layout (bf16): WhT_l @ [0:16, 16l), UT_l @ [0:16, 48+16l),
            # Wb_l @ [0:1, 96+16l), We_l[:, c] (row) @ [0:1, 144+(3l+c)*16)
            def WhT(l):
                return cb_t[:, 16 * l:16 * l + 16]

            def UT(l):
                return cb_t[0:HID, 48 + 16 * l:64 + 16 * l]

            def Wb(l):
                return cb_t[0:1, 96 + 16 * l:112 + 16 * l]

            def WeRow(l, c):
                o = 144 + (3 * l + c) * 16
                return cb_t[0:1, o:o + 16]

            def UbCol(l):
                return cf_t[:, l:l + 1]

            # --- phase A: stream edges, esum + (msg0 with folded deg) ---
            esum_ps = [
                ps_acc.tile([1, 512], f32, tag=f"esum{c}", name=f"esum_ps{c}")
                for c in range(EDIM)
            ]
            msg_ps = ps_msg.tile([M0, 512], f32, tag="msg0")

            hw_tiles = [hw0_t, None, None]

            for ch in range(CHUNKS):
                csl = slice(ch * CW, (ch + 1) * CW)
                nc.sync.dma_start(adj_t[:, csl], adj_d[ch])
                e_t = estream.tile([128, EDIM * CW], bf16)
                nc.sync.dma_start(
                    e_t[:].rearrange("p (c w) -> p c w", c=EDIM),
                    ef_d[ch].rearrange("c p w -> p c w"),
                )
                m_t = mpool.tile([128, EDIM * CW], bf16)
                for c in range(EDIM):
                    wsl = slice(c * CW, (c + 1) * CW)
                    nc.vector.tensor_tensor(m_t[:, wsl], adj_t[:, csl],
                                            e_t[:, wsl], OP.mult)
                    for bb in range(BPC):
                        b = ch * BPC + bb
                        nc.tensor.matmul(
                            esum_ps[c][:], ones_col[:],
                            m_t[:, c * CW + bb * 512:c * CW + (bb + 1) * 512],
                            start=(b == 0), stop=(b == NB - 1),
                        )
                for bb in range(BPC):
                    b = ch * BPC + bb
                    nc.tensor.matmul(
                        msg_ps[:], hw0_t[:, b * M0:(b + 1) * M0],
                        adj_t[:, b * 512:(b + 1) * 512],
                        start=(b == 0), stop=False,
                    )

            esum_sb = [
                small.tile([1, 512], bf16, tag=f"esum_sb{c}", name=f"esum_sb{c}")
                for c in range(EDIM)
            ]
            nc.scalar.copy(esum_sb[0][:], esum_ps[0][:])
            nc.vector.tensor_copy(esum_sb[1][:], esum_ps[1][:])
            nc.scalar.copy(esum_sb[2][:], esum_ps[2][:])
            degT = small.tile([1, 512], bf16)
            nc.vector.tensor_copy(degT[:], msg_ps[32:33, :])

            # --- phase B: layers ---
            hprev_loc = h0loc_t
            for l in range(LAYERS):
                if l > 0:
                    msg_ps = ps_msg.tile([HID, 512], f32, tag="msg")
                    for b in range(NB):
                        nc.tensor.matmul(
                            msg_ps[:], hw_tiles[l][:, b * HID:(b + 1) * HID],
                            adj_t[:, b * 512:(b + 1) * 512],
                            start=(b == 0), stop=False,
                        )
                for c in range(EDIM):
                    nc.tensor.matmul(msg_ps[0:HID, :], WeRow(l, c), esum_sb[c][:],
                                     start=False, stop=False)
                nc.tensor.matmul(msg_ps[0:HID, :], Wb(l), degT[:],
                                 start=False, stop=True)

                xT = small.tile([HID, 512], bf16, tag="xT", bufs=2)
                nc.vector.tensor_tensor(xT[:], hprev_loc[:], msg_ps[0:HID, :], OP.add)

                h_ps = ps_h.tile([HID, 512], f32, tag="h")
                nc.tensor.matmul(h_ps[:], UT(l), xT[:], start=True, stop=True)

                if l < LAYERS - 1:
                    hloc_f = small.tile([HID, 512], f32, tag="hlocf", bufs=2)
                    nc.scalar.activation(hloc_f[:], h_ps[:], AF.Identity,
                                         bias=UbCol(l), scale=1.0)
                    hloc_b = small.tile([HID, 512], bf16, tag="hlocb", bufs=2)
                    nc.vector.tensor_scalar_add(hloc_b[:], h_ps[:], UbCol(l))
                    cc_in = dram.tile([HID, 512], bf16, tag="cc_in")
                    nc.sync.dma_start(cc_in[:], hloc_b[:])
                    no_cc = bool(os.environ.get("DMPNN_NO_CC"))
                    cc_out = dram.tile([NCORES * HID, 512], bf16, tag="cc_out",
                                       addr_space="Local" if no_cc else "Shared")
                    if no_cc:
                        for r in range(NCORES):
                            nc.sync.dma_start(cc_out[r * HID:(r + 1) * HID, :],
                                              cc_in[:])
                    else:
                        nc.gpsimd.collective_compute(
                            "AllGather", mybir.AluOpType.bypass,
                            replica_groups=rg,
                            ins=[cc_in[:]], outs=[cc_out[:]],
                        )
                    hT_full = hbuf.tile([HID, N], bf16, tag="hfull")
                    nc.sync.dma_start(
                        hT_full[:].rearrange("h (r j) -> h r j", r=NCORES),
                        cc_out[:].rearrange("(r h) j -> h r j", r=NCORES),
                    )
                    hw_ps = ps_hw.tile([128, NB * HID], f32, tag="hw")
                    for b in range(NB):
                        nc.tensor.matmul(
                            hw_ps[:, b * HID:(b + 1) * HID],
                            hT_full[:, b * 128:(b + 1) * 128],
                            WhT(l + 1),
                            start=True, stop=True,
                        )
                    hw_next = hwp.tile([128, NB * HID], bf16, tag=f"hw{l + 1}")
                    nc.vector.tensor_copy(hw_next[:], hw_ps[:])
                    hw_tiles[l + 1] = hw_next
                    hprev_loc = hloc_f
                else:
                    gpart = small.tile([HID, 1], f32)
                    nc.vector.reduce_sum(gpart[:], h_ps[:], axis=AX.X)
                    nc.sync.dma_start(out_d[:], gpart[:])

    nc.compile()
    return nc


def _get_module():
    if "nc" not in _CACHE:
        _CACHE["nc"] = _build_module()
    return _CACHE["nc"]


def _shard_cols(a2d):
    """[4096, 512] -> [CHUNKS, 128, CW] bf16 (block-of-128 layout)."""
    a = np.asarray(a2d).astype(BF16)
    a = a.reshape(CHUNKS, BPC, 128, 512).transpose(0, 2, 1, 3)
    return np.ascontiguousarray(a.reshape(CHUNKS, 128, CW))


def kernel(node_features, edge_features, adj_matrix,
           emb_w, emb_b, W_h, W_e, W_b, U_w, U_b, ro_w, ro_b):
    global LAST_EXEC_TIME_NS, LAST_RESULTS
    from concourse.bass_utils import run_bass_kernel_spmd

    node_features = np.asarray(node_features, dtype=np.float32)
    edge_features = np.asarray(edge_features, dtype=np.float32)
    adj_matrix = np.asarray(adj_matrix, dtype=np.float32)
    W_h = np.asarray(W_h, dtype=np.float32)
    W_e = np.asarray(W_e, dtype=np.float32)
    W_b = np.asarray(W_b, dtype=np.float32)
    U_w = np.asarray(U_w, dtype=np.float32)
    U_b = np.asarray(U_b, dtype=np.float32)

    # tiny host-side preprocessing (node embedding + layer-0 hW)
    h0 = node_features @ np.asarray(emb_w, np.float32).T + np.asarray(emb_b, np.float32)
    hw0_full = h0 @ W_h[0].T                                   # [N, HID]
    hw0 = np.zeros((NB, 128, M0), dtype=BF16)
    hw0[:, :, 0:HID] = hw0_full.reshape(NB, 128, HID).astype(BF16)
    hw0[:, :, 32] = 1.0
    hw0 = np.ascontiguousarray(hw0.transpose(1, 0, 2).reshape(128, NB * M0))

    cb = np.zeros((HID, 18 * HID), dtype=BF16)
    cf = np.zeros((HID, LAYERS), dtype=np.float32)
    for l in range(LAYERS):
        cb[:, 16 * l:16 * l + 16] = W_h[l].T.astype(BF16)
        cb[0:HID, 48 + 16 * l:64 + 16 * l] = U_w[l].T.astype(BF16)
        cb[0:1, 96 + 16 * l:112 + 16 * l] = W_b[l][None, :].astype(BF16)
        for c in range(EDIM):
            o = 144 + (3 * l + c) * 16
            cb[0:1, o:o + 16] = W_e[l][:, c][None, :].astype(BF16)
        cf[:, l] = U_b[l]

    ebf = edge_features.astype(BF16)
    h0_bf = h0.astype(BF16)

    in_maps = []
    for k in range(NCORES):
        cols = slice(k * J, (k + 1) * J)
        adj_s = _shard_cols(adj_matrix[:, cols])
        ef_s = np.ascontiguousarray(np.stack(
            [_shard_cols(ebf[:, cols, c]) for c in range(EDIM)]
        ).transpose(1, 0, 2, 3))              # [CHUNKS, EDIM, 128, CW]
        h0loc = np.ascontiguousarray(h0_bf[cols, :].T)
        in_maps.append({
            "adj": adj_s, "ef": ef_s, "h0loc": h0loc,
            "hw0": hw0, "cb": cb, "cf": cf,
        })

    nc = _get_module()
    res = run_bass_kernel_spmd(nc, in_maps, core_ids=list(range(NCORES)))
    LAST_EXEC_TIME_NS = res.exec_time_ns
    LAST_RESULTS = res

    graph_rep = np.zeros(HID, dtype=np.float64)
    for k in range(NCORES):
        graph_rep += res.results[k]["out_p"][:, 0].astype(np.float64)
    graph_rep += N * U_b[LAYERS - 1].astype(np.float64)  # last-layer bias
    out = graph_rep.astype(np.float32) @ np.asarray(ro_w, np.float32).T \
        + np.asarray(ro_b, np.float32)
    return out.astype(np.float32)
